# revision 1
# baseline (speedup 1.0000x reference)
"""ChirpletSynth Trainium2 kernel.

out[b, n] = sin(2*pi*phi) * fm * exp(-(ws*inv)^2) * sin(2*pi*am*0.5*t)
  phi = (F0/(fm*ln2)) * (2^(fm*t) - 1)

Sharding: each of the 8 cores computes the full batch (256) for a
contiguous 8192-sample slice of n. Layout on core: partition = batch
(2 groups of 128), free dim = n-chunk.

Per (group, chunk) op graph:
  ACT : e     = exp(fm_ln2 * t)                      (scale per-partition)
  DVE : ry    = z - round(z),  z = e*c_phi - c_lo    (custom fused op)
  ACT : car   = sin(2pi * ry)
  DVE : rq    = q - round(q),  q = t*am_half          (same custom op)
  ACT : mod   = sin(2pi * rq)
  ACT : winfm = exp(neg_inv2*ws2 + ln_fm)            (scale+bias per-partition)
  POOL: p1    = car * mod
  DVE : o     = p1 * winfm
round() via the float32 magic constant M=1.5*2^23; sin args are then in
[-pi, pi], inside the ACT Sin table's valid range (|x| < 4).
"""

import math
import os

import numpy as np

P = 128
B = 256
N = 65536
NCORES = 8
NSLICE = N // NCORES  # 8192
NGROUPS = B // P  # 2

SR = 44100.0
F0 = 440.0
SIGMA0 = 0.1
BW_N = 44100
LN2 = math.log(2.0)
TWO_PI = 2.0 * math.pi
MAGIC = 12582912.0  # 1.5 * 2**23

f32 = np.float32

_OP = None
_OP2 = None
_NC_CACHE = {}
LAST_RESULT = None
VSUB = 128  # inner split of n for the separable-exp trick: n = 128*U + V


def _register_chirp_op():
    """Register the fused range-reduction op:  out = z - round(z),
    z = in0*s0 - s1  (round via +M/-M magic, M passed as imm2)."""
    global _OP
    if _OP is not None:
        return _OP
    import concourse.dve_ops as D
    from concourse.dve_spec import Spec, Src0, C0, C1, C2, lower, _has_src1
    from concourse.dve_uop import DveOpSpec

    name = "CHIRP_RANGE_RED"
    for op in D.OPS:
        if op.name == name:
            _OP = op
            return op

    z = Src0 * C0 - C1
    body = z - ((z + C2) - C2)

    def _ref(in0, in1, s0, s1, imm2):
        zz = (in0.astype(np.float32) * np.float32(1) * s0).astype(np.float32)
        zz = (zz - s1).astype(np.float32)
        u = (zz + np.float32(imm2)).astype(np.float32)
        r = (u - np.float32(imm2)).astype(np.float32)
        return (zz - r).astype(np.float32)

    spec = Spec(body=body, reference=_ref)
    row = D._CUSTOM_DVE_ROW_BASE + len(D.OPS)
    assert row < 0x20, "custom-DVE opcode rows exhausted"
    D._SUB_OPCODE_FOR_NAME[name] = row
    shas = {}
    for ver in ("v3", "v4"):
        tmp = DveOpSpec(
            name=name, opcode=row, uops=lower(spec, ver=ver), rd1_en=_has_src1(spec)
        )
        shas[ver] = tmp.sha(ver)
    op = D.DveOp(name, spec, subdim=False, uops_sha=shas)
    D.OPS.append(op)
    D.CUSTOM_DVE_SPECS[name] = spec
    _OP = op
    return op


def _register_chirp_exp_op():
    """2-stream fused op:  w = in0*in1 - s0 ;  out = w - round(w)
    (round via the magic constant passed as the s1 literal).  in0/in1 are
    broadcast APs of the separable exp factors E1' = c_phi*exp(outer),
    E2 = exp(inner), so this one instruction computes the chirp phase AND
    its range reduction with no ACT exp pass."""
    global _OP2
    if _OP2 is not None:
        return _OP2
    import concourse.dve_ops as D
    from concourse.dve_spec import Spec, Src0, Src1, C0, C1, lower, _has_src1
    from concourse.dve_uop import DveOpSpec

    name = "CHIRP_EXP_RED"
    for op in D.OPS:
        if op.name == name:
            _OP2 = op
            return op

    w = Src0 * Src1 - C0
    body = w - ((w + C1) - C1)

    def _ref(in0, in1, s0, s1, imm2):
        ww = (in0.astype(np.float32) * in1.astype(np.float32)).astype(np.float32)
        ww = (ww - s0).astype(np.float32)
        u = (ww + np.float32(s1)).astype(np.float32)
        r = (u - np.float32(s1)).astype(np.float32)
        return (ww - r).astype(np.float32)

    spec = Spec(body=body, reference=_ref)
    row = D._CUSTOM_DVE_ROW_BASE + len(D.OPS)
    assert row < 0x20, "custom-DVE opcode rows exhausted"
    D._SUB_OPCODE_FOR_NAME[name] = row
    shas = {}
    for ver in ("v3", "v4"):
        tmp = DveOpSpec(
            name=name, opcode=row, uops=lower(spec, ver=ver), rd1_en=_has_src1(spec)
        )
        shas[ver] = tmp.sha(ver)
    op = D.DveOp(name, spec, subdim=False, uops_sha=shas)
    D.OPS.append(op)
    D.CUSTOM_DVE_SPECS[name] = spec
    _OP2 = op
    return op


_OP3 = None


def _register_chirp_add_op():
    """2-stream additive fused op:  w = in0 + in1 ;  out = w - round(w)
    (round via the magic constant in the s1 literal). in0/in1 are broadcast
    APs of the additive modulator-phase split QA[b,U] + QB[b,V], so this one
    instruction computes the modulator phase AND its range reduction with no
    iota tile or broadcast DMA."""
    global _OP3
    if _OP3 is not None:
        return _OP3
    import concourse.dve_ops as D
    from concourse.dve_spec import Spec, Src0, Src1, C1, lower, _has_src1
    from concourse.dve_uop import DveOpSpec

    name = "CHIRP_ADD_RED"
    for op in D.OPS:
        if op.name == name:
            _OP3 = op
            return op

    w = Src0 + Src1
    body = w - ((w + C1) - C1)

    def _ref(in0, in1, s0, s1, imm2):
        ww = (in0.astype(np.float32) + in1.astype(np.float32)).astype(np.float32)
        u = (ww + np.float32(s1)).astype(np.float32)
        r = (u - np.float32(s1)).astype(np.float32)
        return (ww - r).astype(np.float32)

    spec = Spec(body=body, reference=_ref)
    row = D._CUSTOM_DVE_ROW_BASE + len(D.OPS)
    assert row < 0x20, "custom-DVE opcode rows exhausted"
    D._SUB_OPCODE_FOR_NAME[name] = row
    shas = {}
    for ver in ("v3", "v4"):
        tmp = DveOpSpec(
            name=name, opcode=row, uops=lower(spec, ver=ver), rd1_en=_has_src1(spec)
        )
        shas[ver] = tmp.sha(ver)
    op = D.DveOp(name, spec, subdim=False, uops_sha=shas)
    D.OPS.append(op)
    D.CUSTOM_DVE_SPECS[name] = spec
    _OP3 = op
    return op


def _build_nc_v2(chunk_f, qb, repeat=1):
    """v2: t from on-device iota (no broadcast DMA), window arg via PE
    outer-product into PSUM (no ws2 broadcast), concatenated sin pass,
    exp/sin emitted in batches of `qb` iterations to amortize ACT
    table loads."""
    import concourse.bass as bass  # noqa: F401
    import concourse.mybir as mybir
    from concourse import bacc
    from concourse.tile import TileContext, add_dep_helper

    AFT = mybir.ActivationFunctionType
    dt = mybir.dt
    alu = mybir.AluOpType
    op = _register_chirp_op()
    op2 = _register_chirp_exp_op()
    op3 = _register_chirp_add_op()

    NU = NSLICE // VSUB  # U values per slice (64)
    fp16 = bool(int(os.environ.get("CHIRP_FP16", "0")))
    odt = dt.float16 if fp16 else dt.float32
    nc = bacc.Bacc(None, target_bir_lowering=False, debug=False)
    scal = nc.declare_dram_parameter("scal", [B, 16], dt.float32, isOutput=False)
    arange_row = nc.declare_dram_parameter(
        "arange_row", [1, chunk_f], dt.float32, isOutput=False
    )
    e1 = nc.declare_dram_parameter("e1", [B, NU], dt.float32, isOutput=False)
    e2 = nc.declare_dram_parameter("e2", [B, VSUB], dt.float32, isOutput=False)
    ws2hl = nc.declare_dram_parameter(
        "ws2hl", [2, NSLICE], dt.bfloat16, isOutput=False
    )
    out = nc.declare_dram_parameter("out", [B, NSLICE], odt, isOutput=True)

    n_chunks = NSLICE // chunk_f
    Fc = chunk_f
    INV_SR = float(np.float32(1.0) / np.float32(SR))

    if int(os.environ.get("CHIRP_GMAJOR", "0")):
        iters = [(c, g) for g in range(NGROUPS) for c in range(n_chunks)]
    else:
        iters = [(c, g) for c in range(n_chunks) for g in range(NGROUPS)]

    with TileContext(nc) as tc:
        with (
            tc.tile_pool(name="consts", bufs=1) as cpool,
            tc.tile_pool(name="tt", bufs=n_chunks) as tpool,
            tc.tile_pool(name="keep", bufs=qb) as kpool,
            tc.tile_pool(name="work", bufs=2) as wpool,
            tc.tile_pool(name="psum", bufs=2, space="PSUM") as ppool,
        ):
            # iota broadcast first: it gates every modulator custom
            iota_t = cpool.tile([P, Fc], dt.float32, tag="iota", name="iota")
            nc.sync.dma_start(
                out=iota_t[:], in_=arange_row[0:1, :].to_broadcast((P, Fc))
            )
            scal_t = []
            e1_t = []
            e2_t = []
            for g in range(NGROUPS):
                st = cpool.tile([P, 16], dt.float32, tag=f"scal{g}", name=f"scal{g}")
                nc.sync.dma_start(out=st[:], in_=scal[g * P : (g + 1) * P, :])
                scal_t.append(st)
                e1g = cpool.tile([P, NU], dt.float32, tag=f"e1{g}", name=f"e1{g}")
                nc.sync.dma_start(out=e1g[:], in_=e1[g * P : (g + 1) * P, :])
                e1_t.append(e1g)
                e2g = cpool.tile([P, VSUB], dt.float32, tag=f"e2{g}", name=f"e2{g}")
                nc.sync.dma_start(out=e2g[:], in_=e2[g * P : (g + 1) * P, :])
                e2_t.append(e2g)
            ones_bf = cpool.tile([2, P], dt.bfloat16, tag="ones", name="ones")
            nc.gpsimd.memset(ones_bf[:], 1.0)

            ws2_tiles = {}
            for c in range(n_chunks // 2):
                wr = tpool.tile([2, Fc], dt.bfloat16, tag="ws2r", name="ws2r")
                nc.sync.dma_start(out=wr[:], in_=ws2hl[:, c * Fc : (c + 1) * Fc])
                ws2_tiles[c] = wr

            NUC = Fc // VSUB  # U values per chunk (16)
            # chunk c>=n_chunks//2 mirrors chunk (n_chunks-1-c): its window is
            # a reversed read of the stored winfm (host maps chunk offsets)
            half = n_chunks // 2
            all_iters = iters * repeat
            prev_last_sin = None
            winfm_store = {}
            mul_idx = 0
            for bstart in range(0, len(all_iters), qb):
                batch = all_iters[bstart : bstart + qb]
                stage = {}
                winfm_instrs = []
                sin_instrs = []
                for bi, (c, g) in enumerate(batch):
                    st = scal_t[g]

                    if c < half:
                        wr = ws2_tiles[c]
                        # ws2 broadcast into PSUM: ones.T @ [ws2_hi; ws2_lo]
                        w2ps = ppool.tile(
                            [P, Fc], dt.float32, tag="w2ps", name="w2ps"
                        )
                        for s in range(0, Fc, 512):
                            nc.tensor.matmul(
                                w2ps[:, s : s + 512],
                                ones_bf[:],
                                wr[:, s : s + 512],
                                start=True,
                                stop=True,
                            )
                        winfm = kpool.tile(
                            [P, Fc], odt, tag="winfm", name="winfm",
                            bufs=2 * NGROUPS,
                        )
                        wi = nc.scalar.activation(
                            winfm[:], w2ps[:], AFT.Exp,
                            scale=st[:, 4:5], bias=st[:, 5:6],
                        )
                        winfm_instrs.append(wi)
                        winfm_store[(c, g)] = winfm
                        winfm_ap = winfm[:]
                    else:
                        winfm_ap = winfm_store[(n_chunks - 1 - c, g)][:, ::-1]

                    rr = kpool.tile(
                        [P, 2 * Fc], dt.float32, tag="rr", name="rr",
                        bufs=int(os.environ.get("CHIRP_RRB", "5")),
                    )
                    # modulator phase + reduction: q = am_half/SR*iota + qoff_c
                    nc.vector._custom_dve(
                        op, out=rr[:, Fc : 2 * Fc], in0=iota_t[:], s0=st[:, 6:7],
                        s1=st[:, 8 + c : 9 + c], imm2=MAGIC,
                    )
                    # carrier phase + range reduction fused: separable exp
                    in0 = e1_t[g][:, c * NUC : (c + 1) * NUC, None].broadcast_to(
                        (P, NUC, VSUB)
                    )
                    in1 = e2_t[g][:, None, :].broadcast_to((P, NUC, VSUB))
                    ry = rr[:, 0:Fc].rearrange("p (u v) -> p u v", v=VSUB)
                    nc.vector._custom_dve(
                        op2, out=ry, in0=in0, in1=in1, s0=st[:, 2:3], s1=MAGIC
                    )
                    stage[(c, g)] = (winfm_ap, rr)

                for bi, (c, g) in enumerate(batch):
                    winfm_ap, rr = stage[(c, g)]
                    sc = wpool.tile(
                        [P, 2 * Fc], odt, tag="sc", name="sc",
                        bufs=int(os.environ.get("CHIRP_SCB", "2")),
                    )
                    si = nc.scalar.activation(sc[:], rr[:], AFT.Sin, scale=TWO_PI)
                    sin_instrs.append(si)
                    p1 = wpool.tile(
                        [P, Fc], odt, tag="p1", name="p1",
                        bufs=int(os.environ.get("CHIRP_P1B", "3")),
                    )
                    nlb = int(os.environ.get("CHIRP_NLB", "1"))
                    last_batch = bstart + nlb * qb >= len(all_iters)
                    if last_batch:
                        # split the mul chains across DVE and Pool in speed
                        # ratio so the tail after the last sins stays short
                        h = int(os.environ.get("CHIRP_H", "1216"))
                        nc.vector.tensor_mul(
                            p1[:, 0:h], sc[:, 0:h], sc[:, Fc : Fc + h]
                        )
                        nc.gpsimd.tensor_mul(
                            p1[:, h:Fc], sc[:, h:Fc], sc[:, Fc + h : 2 * Fc]
                        )
                        nc.vector.tensor_mul(
                            p1[:, 0:h], p1[:, 0:h], winfm_ap[:, 0:h]
                        )
                        nc.gpsimd.tensor_mul(
                            p1[:, h:Fc], p1[:, h:Fc], winfm_ap[:, h:Fc]
                        )
                        do_dma = True
                    else:
                        do_dma = True
                        scheme = int(os.environ.get("CHIRP_MULS", "8"))
                        if scheme == 0:
                            p1e = nc.gpsimd
                            oe = nc.gpsimd if mul_idx % 3 == 2 else nc.vector
                        elif scheme == 1:
                            p1e = nc.gpsimd
                            oe = nc.vector if mul_idx % 4 == 0 else nc.gpsimd
                        elif scheme == 2:
                            p1e = nc.vector if mul_idx % 2 == 0 else nc.gpsimd
                            oe = nc.gpsimd if mul_idx % 2 == 0 else nc.vector
                        elif scheme == 6:
                            p1e = nc.gpsimd
                            oe = nc.gpsimd if mul_idx in (1, 4) else nc.vector
                        elif scheme == 7:
                            p1e = nc.gpsimd
                            oe = nc.gpsimd if mul_idx in (1,) else nc.vector
                        elif scheme == 8:
                            idxs = tuple(int(x) for x in os.environ.get(
                                "CHIRP_POOLO", "1").split(",") if x != "")
                            p1e = nc.gpsimd
                            oe = nc.gpsimd if mul_idx in idxs else nc.vector
                        else:
                            p1e = nc.gpsimd
                            oe = nc.vector
                        p1e.tensor_mul(p1[:], sc[:, 0:Fc], sc[:, Fc : 2 * Fc])
                        if int(os.environ.get("CHIRP_OSPLIT", "0")):
                            hh = int(os.environ.get("CHIRP_H", "1216"))
                            nc.vector.tensor_mul(
                                p1[:, 0:hh], p1[:, 0:hh], winfm_ap[:, 0:hh]
                            )
                            nc.gpsimd.tensor_mul(
                                p1[:, hh:Fc], p1[:, hh:Fc], winfm_ap[:, hh:Fc]
                            )
                        else:
                            oe.tensor_mul(p1[:], p1[:], winfm_ap)
                        mul_idx += 1
                    if do_dma:
                        nc.sync.dma_start(
                            out=out[g * P : (g + 1) * P, c * Fc : (c + 1) * Fc],
                            in_=p1[:],
                        )

                # pin ACT order: all winfms (exp table) before all sins
                # (trig table) within a batch, batches in sequence
                if winfm_instrs:
                    if prev_last_sin is not None:
                        for wi in winfm_instrs:
                            add_dep_helper(
                                wi.ins, prev_last_sin.ins, False,
                                "act-table phase order",
                            )
                    for si in sin_instrs:
                        add_dep_helper(
                            si.ins, winfm_instrs[-1].ins, False,
                            "act-table phase order",
                        )
                prev_last_sin = sin_instrs[-1]
    nc.compile()
    return nc


def _build_nc(chunk_f):
    import concourse.bass as bass  # noqa: F401
    import concourse.mybir as mybir
    from concourse import bacc
    from concourse.tile import TileContext

    AFT = mybir.ActivationFunctionType
    dt = mybir.dt
    op = _register_chirp_op()

    nc = bacc.Bacc(None, target_bir_lowering=False, debug=False)
    scal = nc.declare_dram_parameter("scal", [B, 8], dt.float32, isOutput=False)
    t_row = nc.declare_dram_parameter("t_row", [1, NSLICE], dt.float32, isOutput=False)
    ws2_row = nc.declare_dram_parameter(
        "ws2_row", [1, NSLICE], dt.float32, isOutput=False
    )
    out = nc.declare_dram_parameter("out", [B, NSLICE], odt, isOutput=True)

    n_chunks = NSLICE // chunk_f
    Fc = chunk_f

    with TileContext(nc) as tc:
        with (
            tc.tile_pool(name="consts", bufs=1) as cpool,
            tc.tile_pool(name="bcast", bufs=2) as bpool,
            tc.tile_pool(name="work", bufs=2) as wpool,
        ):
            scal_t = []
            for g in range(NGROUPS):
                st = cpool.tile([P, 8], dt.float32, tag=f"scal{g}", name=f"scal{g}")
                nc.sync.dma_start(out=st[:], in_=scal[g * P : (g + 1) * P, :])
                scal_t.append(st)

            for c in range(n_chunks):
                sl = slice(c * Fc, (c + 1) * Fc)
                tb = bpool.tile([P, Fc], dt.float32, tag="tb", name="tb")
                nc.sync.dma_start(out=tb[:], in_=t_row[0:1, sl].to_broadcast((P, Fc)))
                wsb = bpool.tile([P, Fc], dt.float32, tag="wsb", name="wsb")
                nc.sync.dma_start(
                    out=wsb[:], in_=ws2_row[0:1, sl].to_broadcast((P, Fc))
                )
                for g in range(NGROUPS):
                    st = scal_t[g]
                    fm_ln2 = st[:, 0:1]
                    c_phi = st[:, 1:2]
                    c_lo = st[:, 2:3]
                    am_half = st[:, 3:4]
                    neg_inv2 = st[:, 4:5]
                    ln_fm = st[:, 5:6]

                    e = wpool.tile([P, Fc], dt.float32, tag="e", name="e")
                    nc.scalar.activation(e[:], tb[:], AFT.Exp, scale=fm_ln2)
                    winfm = wpool.tile([P, Fc], dt.float32, tag="winfm", name="winfm")
                    nc.scalar.activation(
                        winfm[:], wsb[:], AFT.Exp, scale=neg_inv2, bias=ln_fm
                    )

                    ry = wpool.tile([P, Fc], dt.float32, tag="ry", name="ry")
                    nc.vector._custom_dve(
                        op, out=ry[:], in0=e[:], s0=c_phi, s1=c_lo, imm2=MAGIC
                    )
                    rq = wpool.tile([P, Fc], dt.float32, tag="rq", name="rq")
                    nc.vector._custom_dve(
                        op, out=rq[:], in0=tb[:], s0=am_half, s1=0.0, imm2=MAGIC
                    )

                    car = wpool.tile([P, Fc], dt.float32, tag="car", name="car")
                    nc.scalar.activation(car[:], ry[:], AFT.Sin, scale=TWO_PI)
                    mod = wpool.tile([P, Fc], dt.float32, tag="mod", name="mod")
                    nc.scalar.activation(mod[:], rq[:], AFT.Sin, scale=TWO_PI)

                    p1 = wpool.tile([P, Fc], dt.float32, tag="p1", name="p1")
                    nc.gpsimd.tensor_mul(p1[:], car[:], mod[:])
                    o = wpool.tile([P, Fc], dt.float32, tag="o", name="o")
                    nc.vector.tensor_mul(o[:], p1[:], winfm[:])

                    nc.sync.dma_start(out=out[g * P : (g + 1) * P, sl], in_=o[:])
    nc.compile()
    return nc


def _host_params(theta_am, theta_fm):
    """Per-batch scalars, float32 with rounding mirroring the reference."""
    am_lo, am_hi = f32(math.log2(4.0)), f32(math.log2(16.0))
    fm_lo, fm_hi = f32(math.log2(0.5)), f32(math.log2(4.0))
    am = np.exp2(theta_am * (am_hi - am_lo) + am_lo).astype(f32)
    fm = np.exp2(theta_fm * (fm_hi - fm_lo) + fm_lo).astype(f32)

    fm_ln2 = (fm * f32(LN2)).astype(f32)
    c_phi = (f32(F0) / fm_ln2).astype(f32)
    c_hi = np.rint(c_phi.astype(np.float64)).astype(f32)
    c_lo = (c_phi - c_hi).astype(f32)  # exact
    am_half = (am * f32(0.5)).astype(f32)
    inv_s = (
        f32(1.0)
        / (np.abs(f32(SIGMA0 * BW_N) / fm).astype(f32) * f32(math.sqrt(2.0)))
    ).astype(f32)
    neg_inv2 = (-(inv_s * inv_s)).astype(f32)
    ln_fm = np.log(fm.astype(np.float64)).astype(f32)

    scal = np.zeros((B, 16), dtype=f32)
    scal[:, 0] = fm_ln2
    scal[:, 1] = c_phi
    scal[:, 2] = c_lo
    scal[:, 3] = am_half
    scal[:, 4] = neg_inv2
    scal[:, 5] = ln_fm
    scal[:, 6] = (am_half.astype(np.float64) / SR).astype(f32)
    return scal


def chunk_starts(k, chunk_f):
    """Global start index of each on-device chunk for core k. First half of
    the chunks cover the core's slice of the left half of n; the second half
    mirror them on the right, so the gaussian window can be reused reversed."""
    n_chunks = NSLICE // chunk_f
    half = n_chunks // 2
    starts = []
    for c in range(n_chunks):
        if c < half:
            starts.append(k * (NSLICE // 2) + c * chunk_f)
        else:
            starts.append(N - k * (NSLICE // 2) - (n_chunks - c) * chunk_f)
    return starts


def assemble(outs, chunk_f):
    """Gather per-core outputs [B, NSLICE] into the full [B, N]."""
    full = np.empty((B, N), dtype=f32)
    for k, o in enumerate(outs):
        if o.dtype != f32:
            o = o.astype(f32)
        for c, s in enumerate(chunk_starts(k, chunk_f)):
            full[:, s : s + chunk_f] = o[:, c * chunk_f : (c + 1) * chunk_f]
    return full


def make_in_maps(theta_am, theta_fm, version):
    scal = _host_params(theta_am, theta_fm)
    t_full = ((np.arange(N, dtype=f32) - f32(N // 2)) / f32(SR)).astype(f32)
    ws_full = (np.arange(N, dtype=f32) - f32((N - 1) / 2.0)).astype(f32)
    ws2_full = (ws_full * ws_full).astype(f32)

    arange_row = np.arange(int(os.environ.get("CHIRP_F", "2048")), dtype=f32)[None, :]

    # separable exp factors (f64 host precompute):
    #   c_phi*exp(fm_ln2*t[n]) = E1[b, U]*E2[b, V],  n = n0 + VSUB*U + V
    fm_ln2_64 = scal[:, 0].astype(np.float64)
    c_phi_64 = scal[:, 1].astype(np.float64)
    NU = NSLICE // VSUB
    v_idx = np.arange(VSUB, dtype=np.float64)
    e2_arr = np.exp(fm_ln2_64[:, None] * v_idx[None, :] / SR).astype(f32)  # [B, VSUB]
    am_half_all = None  # set below per scal
    qb_arr = None
    import ml_dtypes

    bf16 = ml_dtypes.bfloat16
    ws2_hi = ws2_full.astype(bf16)
    ws2_lo = (ws2_full - ws2_hi.astype(f32)).astype(bf16)

    chunk_f = int(os.environ.get("CHIRP_F", "2048"))
    n_chunks = NSLICE // chunk_f
    half = n_chunks // 2
    NUC = chunk_f // VSUB
    am_half_64 = scal[:, 3].astype(np.float64)
    qb_arr = (am_half_64[:, None] * v_idx[None, :] / SR).astype(f32)  # [B, VSUB]
    in_maps = []
    for k in range(NCORES):
        sl = slice(k * NSLICE, (k + 1) * NSLICE)
        if version == 2:
            starts = chunk_starts(k, chunk_f)
            ws2hl = np.zeros((2, NSLICE), dtype=ws2_hi.dtype)
            for c in range(half):
                s = starts[c]
                ws2hl[0, c * chunk_f : (c + 1) * chunk_f] = ws2_hi[s : s + chunk_f]
                ws2hl[1, c * chunk_f : (c + 1) * chunk_f] = ws2_lo[s : s + chunk_f]
            e1_arr = np.empty((B, NU), dtype=f32)
            qa_arr = np.empty((B, NU), dtype=f32)
            scal_k = scal.copy()
            for c in range(n_chunks):
                n0c = starts[c] - N // 2
                u_idx = n0c + VSUB * np.arange(NUC, dtype=np.float64)
                e1_arr[:, c * NUC : (c + 1) * NUC] = (
                    c_phi_64[:, None]
                    * np.exp(fm_ln2_64[:, None] * u_idx[None, :] / SR)
                ).astype(f32)
                qa_arr[:, c * NUC : (c + 1) * NUC] = (
                    am_half_64[:, None] * u_idx[None, :] / SR
                ).astype(f32)
                scal_k[:, 8 + c] = (-(am_half_64 * n0c) / SR).astype(f32)
            in_maps.append(
                {
                    "scal": scal_k,
                    "arange_row": arange_row,
                    "e1": e1_arr,
                    "e2": e2_arr,
                    "ws2hl": ws2hl,
                }
            )
        else:
            in_maps.append(
                {
                    "scal": scal,
                    "t_row": t_full[None, sl].copy(),
                    "ws2_row": ws2_full[None, sl].copy(),
                }
            )
    return in_maps


def build(version=None, chunk_f=None, qb=None):
    version = int(os.environ.get("CHIRP_V", "2")) if version is None else version
    chunk_f = int(os.environ.get("CHIRP_F", "2048")) if chunk_f is None else chunk_f
    qb = int(os.environ.get("CHIRP_QB", "2")) if qb is None else qb
    key = (version, chunk_f, qb)
    if key not in _NC_CACHE:
        _NC_CACHE[key] = (
            _build_nc_v2(chunk_f, qb) if version == 2 else _build_nc(chunk_f)
        )
    return _NC_CACHE[key], version


def kernel(theta_am_hz_0to1, theta_fm_hz_0to1, seed=None, **_ignored):
    global LAST_RESULT
    from concourse.bass_utils import run_bass_kernel_spmd

    theta_am = np.asarray(theta_am_hz_0to1, dtype=f32)
    theta_fm = np.asarray(theta_fm_hz_0to1, dtype=f32)

    nc, version = build()
    in_maps = make_in_maps(theta_am, theta_fm, version)

    trace = bool(int(os.environ.get("CHIRP_TRACE", "0")))
    res = run_bass_kernel_spmd(
        nc, in_maps, core_ids=list(range(NCORES)), trace=trace
    )
    LAST_RESULT = res
    outs = [r["out"] for r in res.results]
    if version == 2:
        full = assemble(outs, int(os.environ.get("CHIRP_F", "2048")))
    else:
        full = np.concatenate(outs, axis=1)  # [B, N]
    return np.ascontiguousarray(full.reshape(B, 1, N))



# revision 2
# speedup vs baseline: 1.0465x; 1.0465x over previous
"""ChirpletSynth Trainium2 kernel (v3).

out[b, n] = sin(2*pi*phi) * fm * exp(-(ws*inv)^2) * sin(2*pi*am*0.5*t)
  phi = (F0/(fm*ln2)) * (2^(fm*t) - 1)

Sharding: each of the 8 cores computes the full batch (256) for a
contiguous 8192-sample slice of n (mirror-paired chunks so the gaussian
window is computed once and reused reversed). Layout on core:
partition = batch (2 groups of 128), free dim = n-chunk (2048).

Per (group, chunk) tile:
  DVE : ry    = red(E1*E2 - c_lo)            (CHIRP_EXP_RED custom,
                separable exp factors E1[b,u]*E2[b,v], fp32)
  ACT : car   = Sin(2pi * ry)                -> fp16
  modulator, either
    ACT : mod = Sin(sc_p * iota + bi_p)      (phase affine in n; per-chunk
                half-integer offset + sign trick keeps |arg| <= 2.74)
    DVE : p1  = car * mod                    (fp16 tensor mul, 2x mode)
  or fused on DVE:
    DVE : p1  = car * sinpoly5(s0*(J - s1))  (CHIRP_MOD5 custom: scan-based
                index J, deg-5 minimax sin, C3-spilled coefficient)
  ACT : winfm = Exp(neg_inv2*ws2 + ln_fm)    (from PE-broadcast ws2, half the
                chunks; mirrored chunks read it reversed)
  DVE/Pool: o = p1 * winfm                   (fp16)
All Exp activations are emitted before all Sin ones (ACT table loads).
"""

import math
import os

import numpy as np

P = 128
B = 256
N = 65536
NCORES = 8
NSLICE = N // NCORES  # 8192
NGROUPS = B // P  # 2

SR = 44100.0
F0 = 440.0
SIGMA0 = 0.1
BW_N = 44100
LN2 = math.log(2.0)
TWO_PI = 2.0 * math.pi
MAGIC = 12582912.0  # 1.5 * 2**23

VSUB = 128
FC = 2048
N_CHUNKS = NSLICE // FC  # 4
HALF = N_CHUNKS // 2
NU = NSLICE // VSUB  # 64
NUC = FC // VSUB  # 16

# deg-5 minimax of sin(2*pi*y) on |y| <= 0.4365, factored with s^5 = c5:
#   sin(2*pi*y) ~= y'*((y'^2 + B)*y'^2 + A),  y' = s*y
_C1, _C3, _C5 = 6.236727, -39.32464819, 59.29172001
S_NORM = _C5 ** 0.2
A_COEF = _C1 / S_NORM
B_COEF = _C3 / S_NORM ** 3

f32 = np.float32

_OP2 = None
_OP5 = None
_NC_CACHE = {}
LAST_RESULT = None


def _register_chirp_exp_op():
    """w = in0*in1 - s0 ; out = w - round(w) (round via magic constant s1)."""
    global _OP2
    if _OP2 is not None:
        return _OP2
    import concourse.dve_ops as D
    from concourse.dve_spec import Spec, Src0, Src1, C0, C1, lower, _has_src1
    from concourse.dve_uop import DveOpSpec

    name = "CHIRP_EXP_RED"
    for op in D.OPS:
        if op.name == name:
            _OP2 = op
            return op

    w = Src0 * Src1 - C0
    body = w - ((w + C1) - C1)

    def _ref(in0, in1, s0, s1, imm2):
        ww = (in0.astype(np.float32) * in1.astype(np.float32)).astype(np.float32)
        ww = (ww - s0).astype(np.float32)
        u = (ww + np.float32(s1)).astype(np.float32)
        r = (u - np.float32(s1)).astype(np.float32)
        return (ww - r).astype(np.float32)

    spec = Spec(body=body, reference=_ref)
    row = D._CUSTOM_DVE_ROW_BASE + len(D.OPS)
    assert row < 0x20, "custom-DVE opcode rows exhausted"
    D._SUB_OPCODE_FOR_NAME[name] = row
    shas = {}
    for ver in ("v3", "v4"):
        tmp = DveOpSpec(
            name=name, opcode=row, uops=lower(spec, ver=ver), rd1_en=_has_src1(spec)
        )
        shas[ver] = tmp.sha(ver)
    op = D.DveOp(name, spec, subdim=False, uops_sha=shas)
    D.OPS.append(op)
    D.CUSTOM_DVE_SPECS[name] = spec
    _OP2 = op
    return op


def _register_chirp_mod5_op():
    """p1 = in0 * sinpoly5(s0*(J - s1)), J(k) = k+1 via an ADD-scan with the
    offset folded into the scan init. Deg-5 coefficient b rides the C3 spill
    (in1, read once at element 0); a is the imm2 literal."""
    global _OP5
    if _OP5 is not None:
        return _OP5
    import concourse.dve_ops as D
    from concourse.dve_spec import (
        Spec, Src0, C0, C1, C2, C3, Zero, One, scan, lower,
        _has_src1, _spill_c3_to_src1, AluOp,
    )
    from concourse.dve_uop import DveOpSpec

    name = "CHIRP_MOD5"
    for op in D.OPS:
        if op.name == name:
            _OP5 = op
            return op

    J = scan(AluOp.ADD, One, init=Zero - C1)  # J(k) = (k+1) - s1
    y = J * C0
    t = y * y
    m = t + C3
    n = m * t
    n2 = n + C2
    r = n2 * y
    body = _spill_c3_to_src1(r * Src0)

    def _ref(in0, in1, s0, s1, imm2):
        k = np.arange(in0.shape[-1], dtype=np.float32)
        J_ = (k + np.float32(1.0)) - np.float32(s1)
        y_ = (J_ * np.float32(s0)).astype(np.float32)
        t_ = y_ * y_
        b = np.float32(in1.reshape(in1.shape[0], -1)[:, 0:1])
        return (((t_ + b) * t_ + np.float32(imm2)) * y_ * in0.astype(np.float32)
                ).astype(np.float32)

    spec = Spec(body=body, reference=_ref)
    row = D._CUSTOM_DVE_ROW_BASE + len(D.OPS)
    assert row < 0x20, "custom-DVE opcode rows exhausted"
    D._SUB_OPCODE_FOR_NAME[name] = row
    shas = {}
    for ver in ("v3", "v4"):
        tmp = DveOpSpec(
            name=name, opcode=row, uops=lower(spec, ver=ver), rd1_en=_has_src1(spec)
        )
        shas[ver] = tmp.sha(ver)
    op = D.DveOp(name, spec, subdim=False, uops_sha=shas)
    D.OPS.append(op)
    D.CUSTOM_DVE_SPECS[name] = spec
    _OP5 = op
    return op


def _cfg():
    """Engine placement knobs. Tiles are enumerated in TILE_ORDER; opc marks
    tiles whose modulator+p1 run as the fused DVE custom; p1/o strings give
    the mul engine per non-fused tile ('v'=DVE, 'g'=Pool)."""
    order = os.environ.get("CHIRP_ORDER", "")
    if order:
        tiles = [tuple(int(x) for x in t.split(".")) for t in order.split(",")]
    else:
        tiles = [(c, g) for c in range(N_CHUNKS) for g in range(NGROUPS)]
    opc = os.environ.get("CHIRP_OPC", "0,1,2")
    opc_set = {int(x) for x in opc.split(",") if x != ""}
    p1e = os.environ.get("CHIRP_P1E", "ggvvv")  # per non-opc tile, in order
    oe = os.environ.get("CHIRP_OE", "ggggvvvv")  # per tile, in order
    return tiles, opc_set, p1e, oe


def _build_nc_v3():
    import concourse.bass as bass  # noqa: F401
    import concourse.mybir as mybir
    from concourse import bacc
    from concourse.tile import TileContext

    AFT = mybir.ActivationFunctionType
    dt = mybir.dt
    op2 = _register_chirp_exp_op()
    op5 = _register_chirp_mod5_op()

    tiles, opc_set, p1e_s, oe_s = _cfg()

    nc = bacc.Bacc(None, target_bir_lowering=False, debug=False)
    scal = nc.declare_dram_parameter("scal", [B, 32], dt.float32, isOutput=False)
    iota_row = nc.declare_dram_parameter(
        "iota_row", [1, FC], dt.float16, isOutput=False
    )
    e1 = nc.declare_dram_parameter("e1", [B, NU], dt.float32, isOutput=False)
    e2 = nc.declare_dram_parameter("e2", [B, VSUB], dt.float32, isOutput=False)
    ws2hl = nc.declare_dram_parameter("ws2hl", [2, NSLICE], dt.bfloat16, isOutput=False)
    out = nc.declare_dram_parameter("out", [B, NSLICE], dt.float16, isOutput=True)

    with TileContext(nc) as tc:
        with (
            tc.tile_pool(name="consts", bufs=1) as cpool,
            tc.tile_pool(name="keep", bufs=8) as kpool,
            tc.tile_pool(name="work", bufs=3) as wpool,
            tc.tile_pool(name="psum", bufs=2, space="PSUM") as ppool,
        ):
            iota_t = cpool.tile([P, FC], dt.float16, tag="iota", name="iota")
            nc.sync.dma_start(
                out=iota_t[:], in_=iota_row[0:1, :].to_broadcast((P, FC))
            )
            scal_t, e1_t, e2_t = [], [], []
            for g in range(NGROUPS):
                st = cpool.tile([P, 32], dt.float32, tag=f"scal{g}", name=f"scal{g}")
                nc.sync.dma_start(out=st[:], in_=scal[g * P : (g + 1) * P, :])
                scal_t.append(st)
                e1g = cpool.tile([P, NU], dt.float32, tag=f"e1{g}", name=f"e1{g}")
                nc.sync.dma_start(out=e1g[:], in_=e1[g * P : (g + 1) * P, :])
                e1_t.append(e1g)
                e2g = cpool.tile([P, VSUB], dt.float32, tag=f"e2{g}", name=f"e2{g}")
                nc.sync.dma_start(out=e2g[:], in_=e2[g * P : (g + 1) * P, :])
                e2_t.append(e2g)
            ones_bf = cpool.tile([2, P], dt.bfloat16, tag="ones", name="ones")
            nc.gpsimd.memset(ones_bf[:], 1.0)
            ws2_tiles = {}
            for c in range(HALF):
                wr = cpool.tile([2, FC], dt.bfloat16, tag=f"ws2r{c}", name=f"ws2r{c}")
                nc.sync.dma_start(out=wr[:], in_=ws2hl[:, c * FC : (c + 1) * FC])
                ws2_tiles[c] = wr

            # Phase W: all Exp activations first (one act-table phase)
            winfm_store = {}
            for c in range(HALF):
                for g in range(NGROUPS):
                    st = scal_t[g]
                    w2ps = ppool.tile([P, FC], dt.float32, tag="w2ps", name="w2ps")
                    for s in range(0, FC, 512):
                        nc.tensor.matmul(
                            w2ps[:, s : s + 512],
                            ones_bf[:],
                            ws2_tiles[c][:, s : s + 512],
                            start=True,
                            stop=True,
                        )
                    winfm = kpool.tile(
                        [P, FC], dt.float16, tag="winfm", name="winfm", bufs=4
                    )
                    nc.scalar.activation(
                        winfm[:], w2ps[:], AFT.Exp,
                        scale=st[:, 4:5], bias=st[:, 5:6],
                    )
                    winfm_store[(c, g)] = winfm

            # Phase D: all carrier range reductions (DVE front-loaded)
    # ry layout: [P, NUC, VSUB] view of a [P, FC] tile
            ry_store = {}
            for (c, g) in tiles:
                ry = kpool.tile([P, FC], dt.float32, tag="ry", name="ry", bufs=8)
                in0 = e1_t[g][:, c * NUC : (c + 1) * NUC, None].broadcast_to(
                    (P, NUC, VSUB)
                )
                in1 = e2_t[g][:, None, :].broadcast_to((P, NUC, VSUB))
                ryv = ry[:].rearrange("p (u v) -> p u v", v=VSUB)
                nc.vector._custom_dve(
                    op2, out=ryv, in0=in0, in1=in1,
                    s0=scal_t[g][:, 2:3], s1=MAGIC,
                )
                ry_store[(c, g)] = ry

            # Phase S: sins + muls + out DMA per tile
            np1 = 0
            for ti, (c, g) in enumerate(tiles):
                st = scal_t[g]
                if c < HALF:
                    winfm_ap = winfm_store[(c, g)][:]
                else:
                    winfm_ap = winfm_store[(N_CHUNKS - 1 - c, g)][:, ::-1]

                car = wpool.tile([P, FC], dt.float16, tag="car", name="car", bufs=4)
                nc.scalar.activation(
                    car[:], ry_store[(c, g)][:], AFT.Sin, scale=TWO_PI
                )
                p1 = wpool.tile([P, FC], dt.float16, tag="p1", name="p1", bufs=3)
                if ti in opc_set:
                    nc.vector._custom_dve(
                        op5, out=p1[:], in0=car[:], in1=st[:, 26:27],
                        s0=st[:, 16 + c : 17 + c], s1=st[:, 20 + c : 21 + c],
                        imm2=A_COEF,
                    )
                else:
                    mod = wpool.tile(
                        [P, FC], dt.float16, tag="mod", name="mod", bufs=3
                    )
                    nc.scalar.activation(
                        mod[:], iota_t[:], AFT.Sin,
                        scale=st[:, 8 + c : 9 + c], bias=st[:, 12 + c : 13 + c],
                    )
                    eng = nc.vector if p1e_s[np1 % len(p1e_s)] == "v" else nc.gpsimd
                    np1 += 1
                    eng.tensor_mul(p1[:], car[:], mod[:])
                o = wpool.tile([P, FC], dt.float16, tag="o", name="o", bufs=3)
                eng = nc.vector if oe_s[ti % len(oe_s)] == "v" else nc.gpsimd
                eng.tensor_mul(o[:], p1[:], winfm_ap)
                nc.sync.dma_start(
                    out=out[g * P : (g + 1) * P, c * FC : (c + 1) * FC], in_=o[:]
                )
    nc.compile()
    return nc


def chunk_starts(k, chunk_f=FC):
    """Global start index of each on-device chunk for core k. First half of
    the chunks cover the core's slice of the left half of n; the second half
    mirror them on the right, so the gaussian window can be reused reversed."""
    n_chunks = NSLICE // chunk_f
    half = n_chunks // 2
    starts = []
    for c in range(n_chunks):
        if c < half:
            starts.append(k * (NSLICE // 2) + c * chunk_f)
        else:
            starts.append(N - k * (NSLICE // 2) - (n_chunks - c) * chunk_f)
    return starts


def assemble(outs, chunk_f=FC):
    """Gather per-core outputs [B, NSLICE] into the full [B, N]."""
    full = np.empty((B, N), dtype=f32)
    for k, o in enumerate(outs):
        if o.dtype != f32:
            o = o.astype(f32)
        for c, s in enumerate(chunk_starts(k, chunk_f)):
            full[:, s : s + chunk_f] = o[:, c * chunk_f : (c + 1) * chunk_f]
    return full


def _host_params(theta_am, theta_fm):
    am_lo, am_hi = f32(math.log2(4.0)), f32(math.log2(16.0))
    fm_lo, fm_hi = f32(math.log2(0.5)), f32(math.log2(4.0))
    am = np.exp2(theta_am * (am_hi - am_lo) + am_lo).astype(f32)
    fm = np.exp2(theta_fm * (fm_hi - fm_lo) + fm_lo).astype(f32)

    fm_ln2 = (fm * f32(LN2)).astype(f32)
    c_phi = (f32(F0) / fm_ln2).astype(f32)
    c_hi = np.rint(c_phi.astype(np.float64)).astype(f32)
    c_lo = (c_phi - c_hi).astype(f32)
    am_half = (am * f32(0.5)).astype(f32)
    inv_s = (
        f32(1.0)
        / (np.abs(f32(SIGMA0 * BW_N) / fm).astype(f32) * f32(math.sqrt(2.0)))
    ).astype(f32)
    neg_inv2 = (-(inv_s * inv_s)).astype(f32)
    ln_fm = np.log(fm.astype(np.float64)).astype(f32)

    scal = np.zeros((B, 32), dtype=f32)
    scal[:, 2] = c_lo
    scal[:, 4] = neg_inv2
    scal[:, 5] = ln_fm
    scal[:, 26] = B_COEF
    return scal, fm_ln2, c_phi, am_half


def make_in_maps(theta_am, theta_fm):
    scal_base, fm_ln2, c_phi, am_half = _host_params(theta_am, theta_fm)
    fm_ln2_64 = fm_ln2.astype(np.float64)
    c_phi_64 = c_phi.astype(np.float64)
    am_half_64 = am_half.astype(np.float64)
    alpha = am_half_64 / SR  # modulator turns per sample

    ws_full = (np.arange(N, dtype=f32) - f32((N - 1) / 2.0)).astype(f32)
    ws2_full = (ws_full.astype(np.float64) ** 2).astype(f32)
    import ml_dtypes

    bf16 = ml_dtypes.bfloat16
    ws2_hi = ws2_full.astype(bf16)
    ws2_lo = (ws2_full - ws2_hi.astype(f32)).astype(bf16)

    iota_row = np.arange(FC, dtype=np.float16)[None, :]
    v_idx = np.arange(VSUB, dtype=np.float64)
    e2_arr = np.exp(fm_ln2_64[:, None] * v_idx[None, :] / SR).astype(f32)

    in_maps = []
    for k in range(NCORES):
        starts = chunk_starts(k)
        ws2hl = np.zeros((2, NSLICE), dtype=bf16)
        for c in range(HALF):
            s = starts[c]
            ws2hl[0, c * FC : (c + 1) * FC] = ws2_hi[s : s + FC]
            ws2hl[1, c * FC : (c + 1) * FC] = ws2_lo[s : s + FC]
        e1_arr = np.empty((B, NU), dtype=f32)
        scal_k = scal_base.copy()
        for c in range(N_CHUNKS):
            n0c = starts[c] - N // 2
            u_idx = n0c + VSUB * np.arange(NUC, dtype=np.float64)
            e1_arr[:, c * NUC : (c + 1) * NUC] = (
                c_phi_64[:, None]
                * np.exp(fm_ln2_64[:, None] * u_idx[None, :] / SR)
            ).astype(f32)
            # modulator per-chunk params (half-integer offset + sign trick)
            q0 = alpha * n0c  # phase at k=0 of this chunk
            qmid = q0 + alpha * (FC - 1) / 2.0
            r2 = np.round(2.0 * qmid)
            kp = r2 / 2.0
            sigma = 1.0 - 2.0 * (np.abs(r2).astype(np.int64) % 2)
            scal_k[:, 8 + c] = (sigma * TWO_PI * alpha).astype(f32)
            scal_k[:, 12 + c] = (sigma * TWO_PI * (q0 - kp)).astype(f32)
            scal_k[:, 16 + c] = (S_NORM * sigma * alpha).astype(f32)
            scal_k[:, 20 + c] = ((kp - q0) / alpha + 1.0).astype(f32)
        in_maps.append(
            {
                "scal": scal_k,
                "iota_row": iota_row,
                "e1": e1_arr,
                "e2": e2_arr,
                "ws2hl": ws2hl,
            }
        )
    return in_maps


def build():
    key = ("v3", os.environ.get("CHIRP_OPC", ""), os.environ.get("CHIRP_P1E", ""),
           os.environ.get("CHIRP_OE", ""), os.environ.get("CHIRP_ORDER", ""))
    if key not in _NC_CACHE:
        _NC_CACHE[key] = _build_nc_v3()
    return _NC_CACHE[key], 3


def kernel(theta_am_hz_0to1, theta_fm_hz_0to1, seed=None, **_ignored):
    global LAST_RESULT
    from concourse.bass_utils import run_bass_kernel_spmd

    theta_am = np.asarray(theta_am_hz_0to1, dtype=f32)
    theta_fm = np.asarray(theta_fm_hz_0to1, dtype=f32)

    nc, _ = build()
    in_maps = make_in_maps(theta_am, theta_fm)

    trace = bool(int(os.environ.get("CHIRP_TRACE", "0")))
    res = run_bass_kernel_spmd(
        nc, in_maps, core_ids=list(range(NCORES)), trace=trace
    )
    LAST_RESULT = res
    outs = [r["out"] for r in res.results]
    full = assemble(outs)
    return np.ascontiguousarray(full.reshape(B, 1, N))


# revision 14
# speedup vs baseline: 1.1017x; 1.0528x over previous
"""ChirpletSynth Trainium2 kernel (v4: sorted-batch zero-window skipping).

out[b, n] = sin(2*pi*phi) * fm * exp(-(ws*inv)^2) * sin(2*pi*am*0.5*t)
  phi = (F0/(fm*ln2)) * (2^(fm*t) - 1)

The gaussian window exp(-(ws/(std*sqrt2))^2), std = 4410/fm samples, is
negligible beyond |ws| > 4.67*std, so most (batch, chunk) tiles far from
the center are exact zeros. Batches are sorted by fm into 2 groups of
128; for each group only the mirror-pairs of 2048-sample chunks that
intersect the group's support are computed. The needed (pair, group)
units are distributed round-robin over 8 cores; every core runs the
same program over NPAIR slot-pairs, with per-slot constants (scal, e1,
e2, ws2, modulator phases) supplied in the per-core input maps. Host
assembles slots back into the full [B, N] output (and zeros the rest).

Per slot-pair p (tiles L and R, R mirrors L so winfm is reused reversed):
  ACT : winfm = Exp(neg_inv2*ws2 + ln_fm)       (ws2 broadcast, bf16)
  DVE : ry    = red(E1*E2 - c_lo)               (fp32 custom, separable exp)
  ACT : car   = Sin(2pi*ry) -> fp16
  modulator: ACT Sin(sc*iota+bi) + fp16 mul, or fused DVE custom
             car*sinpoly5(s0*(J-s1)) (scan-index J, C3-spilled coeff)
  DVE/Pool: o = p1 * winfm
All Exp activations precede all Sin ones (one act-table switch).
"""

import math
import os

import numpy as np

P = 128
B = 256
N = 65536
NCORES = 8

SR = 44100.0
F0 = 440.0
SIGMA0 = 0.1
BW_N = 44100
LN2 = math.log(2.0)
TWO_PI = 2.0 * math.pi
MAGIC = 12582912.0  # 1.5 * 2**23

VSUB = 128
FC = 2048
NUC = FC // VSUB  # 16
NLEFT = (N // 2) // FC  # 16 left-half chunks; right half mirrors them
SUPPORT_T = 4.67  # |ws| beyond SUPPORT_T*std -> window treated as 0

# deg-5 minimax of sin(2*pi*y) on |y| <= 0.4365, factored with s^5 = c5:
#   sin(2*pi*y) ~= y'*((y'^2 + B)*y'^2 + A),  y' = s*y
_C1, _C3, _C5 = 6.236727, -39.32464819, 59.29172001
S_NORM = _C5 ** 0.2
A_COEF = _C1 / S_NORM
B_COEF = _C3 / S_NORM ** 3

f32 = np.float32

_OP2 = None
_OP5 = None
_NC_CACHE = {}
LAST_RESULT = None
LAST_NC = None


def _register_chirp_exp_op():
    """w = in0*in1 - s0 ; out = w - round(w) (round via magic constant s1)."""
    global _OP2
    if _OP2 is not None:
        return _OP2
    import concourse.dve_ops as D
    from concourse.dve_spec import Spec, Src0, Src1, C0, C1, lower, _has_src1
    from concourse.dve_uop import DveOpSpec

    name = "CHIRP_EXP_RED"
    for op in D.OPS:
        if op.name == name:
            _OP2 = op
            return op

    w = Src0 * Src1 - C0
    body = w - ((w + C1) - C1)

    def _ref(in0, in1, s0, s1, imm2):
        ww = (in0.astype(np.float32) * in1.astype(np.float32)).astype(np.float32)
        ww = (ww - s0).astype(np.float32)
        u = (ww + np.float32(s1)).astype(np.float32)
        r = (u - np.float32(s1)).astype(np.float32)
        return (ww - r).astype(np.float32)

    spec = Spec(body=body, reference=_ref)
    row = D._CUSTOM_DVE_ROW_BASE + len(D.OPS)
    assert row < 0x20, "custom-DVE opcode rows exhausted"
    D._SUB_OPCODE_FOR_NAME[name] = row
    shas = {}
    for ver in ("v3", "v4"):
        tmp = DveOpSpec(
            name=name, opcode=row, uops=lower(spec, ver=ver), rd1_en=_has_src1(spec)
        )
        shas[ver] = tmp.sha(ver)
    op = D.DveOp(name, spec, subdim=False, uops_sha=shas)
    D.OPS.append(op)
    D.CUSTOM_DVE_SPECS[name] = spec
    _OP2 = op
    return op


def _register_chirp_mod5_op():
    """p1 = in0 * sinpoly5(s0*(J - s1)), J(k) = k+1 via an ADD-scan with the
    offset folded into the scan init. Deg-5 coefficient b rides the C3 spill
    (in1, read once at element 0); a is the imm2 literal."""
    global _OP5
    if _OP5 is not None:
        return _OP5
    import concourse.dve_ops as D
    from concourse.dve_spec import (
        Spec, Src0, C0, C1, C2, C3, Zero, One, scan, lower,
        _has_src1, _spill_c3_to_src1, AluOp,
    )
    from concourse.dve_uop import DveOpSpec

    name = "CHIRP_MOD5"
    for op in D.OPS:
        if op.name == name:
            _OP5 = op
            return op

    J = scan(AluOp.ADD, One, init=Zero - C1)  # J(k) = (k+1) - s1
    y = J * C0
    t = y * y
    m = t + C3
    n = m * t
    n2 = n + C2
    r = n2 * y
    body = _spill_c3_to_src1(r * Src0)

    def _ref(in0, in1, s0, s1, imm2):
        k = np.arange(in0.shape[-1], dtype=np.float32)
        J_ = (k + np.float32(1.0)) - np.float32(s1)
        y_ = (J_ * np.float32(s0)).astype(np.float32)
        t_ = y_ * y_
        b = np.float32(in1.reshape(in1.shape[0], -1)[:, 0:1])
        return (((t_ + b) * t_ + np.float32(imm2)) * y_ * in0.astype(np.float32)
                ).astype(np.float32)

    spec = Spec(body=body, reference=_ref)
    row = D._CUSTOM_DVE_ROW_BASE + len(D.OPS)
    assert row < 0x20, "custom-DVE opcode rows exhausted"
    D._SUB_OPCODE_FOR_NAME[name] = row
    shas = {}
    for ver in ("v3", "v4"):
        tmp = DveOpSpec(
            name=name, opcode=row, uops=lower(spec, ver=ver), rd1_en=_has_src1(spec)
        )
        shas[ver] = tmp.sha(ver)
    op = D.DveOp(name, spec, subdim=False, uops_sha=shas)
    D.OPS.append(op)
    D.CUSTOM_DVE_SPECS[name] = spec
    _OP5 = op
    return op


def _build_nc_v4(npair):
    """One program with `npair` slot-pairs (2 tiles each). Per-slot data
    (scal/e1/e2/ws2) comes from DRAM arrays indexed by slot."""
    import concourse.bass as bass  # noqa: F401
    import concourse.mybir as mybir
    from concourse import bacc
    from concourse.tile import TileContext, add_dep_helper

    AFT = mybir.ActivationFunctionType
    dt = mybir.dt
    op2 = _register_chirp_exp_op()
    op5 = _register_chirp_mod5_op()

    ntiles = 2 * npair
    opc = os.environ.get("CHIRP_OPC", "0,1,2")
    opc_set = {int(x) for x in opc.split(",") if x != ""} & set(range(ntiles))
    oe_s = os.environ.get("CHIRP_OE", "gggvvvvv")
    p1e_s = os.environ.get("CHIRP_P1E", "gvvvv")

    nc = bacc.Bacc(None, target_bir_lowering=False, debug=False)
    scal = nc.declare_dram_parameter("scal", [npair * P, 32], dt.float32,
                                     isOutput=False)
    iota_row = nc.declare_dram_parameter("iota_row", [1, FC], dt.float16,
                                         isOutput=False)
    e1 = nc.declare_dram_parameter("e1", [npair * P, 2 * NUC], dt.float32,
                                   isOutput=False)
    e2 = nc.declare_dram_parameter("e2", [npair * P, VSUB], dt.float32,
                                   isOutput=False)
    ws2b = nc.declare_dram_parameter("ws2b", [npair, FC], dt.bfloat16,
                                     isOutput=False)
    out = nc.declare_dram_parameter("out", [npair * P, 2 * FC], dt.float16,
                                    isOutput=True)

    with TileContext(nc) as tc:
        with (
            tc.tile_pool(name="consts", bufs=1) as cpool,
            tc.tile_pool(name="keep", bufs=8) as kpool,
            tc.tile_pool(name="work", bufs=4) as wpool,
        ):
            # const DMAs on SP queue; op2 + exp inputs first, iota last
            scal_t, e1_t, e2_t, ws2_t = [], [], [], []
            for p in range(npair):
                st = cpool.tile([P, 32], dt.float32, tag=f"scal{p}", name=f"scal{p}")
                nc.sync.dma_start(out=st[:], in_=scal[p * P : (p + 1) * P, :])
                scal_t.append(st)
            for p in range(npair):
                wb = cpool.tile([P, FC], dt.bfloat16, tag=f"ws2b{p}",
                                name=f"ws2b{p}")
                nc.sync.dma_start(
                    out=wb[:], in_=ws2b[p : p + 1, :].to_broadcast((P, FC))
                )
                ws2_t.append(wb)
            for p in range(npair):
                e1g = cpool.tile([P, 2 * NUC], dt.float32, tag=f"e1{p}",
                                 name=f"e1{p}")
                nc.sync.dma_start(out=e1g[:], in_=e1[p * P : (p + 1) * P, :])
                e1_t.append(e1g)
                e2g = cpool.tile([P, VSUB], dt.float32, tag=f"e2{p}", name=f"e2{p}")
                nc.sync.dma_start(out=e2g[:], in_=e2[p * P : (p + 1) * P, :])
                e2_t.append(e2g)
            iota_t = cpool.tile([P, FC], dt.float16, tag="iota", name="iota")
            nc.sync.dma_start(
                out=iota_t[:], in_=iota_row[0:1, :].to_broadcast((P, FC))
            )

            # Phase W: all Exps first (one act-table phase)
            winfm_store = []
            exp_instrs = []
            for p in range(npair):
                winfm = kpool.tile([P, FC], dt.float16, tag="winfm", name="winfm",
                                   bufs=npair)
                wi = nc.scalar.activation(
                    winfm[:], ws2_t[p][:], AFT.Exp,
                    scale=scal_t[p][:, 4:5], bias=scal_t[p][:, 5:6],
                )
                exp_instrs.append(wi)
                winfm_store.append(winfm)

            # tiles: ti = 2*p + side (side 0 = left chunk, 1 = mirrored right)
            def tile_pc(ti):
                return ti // 2, ti % 2

            # Phase D: carrier range reductions, opc tiles' ry first
            order = sorted(range(ntiles), key=lambda t: (t not in opc_set, t))
            ry_store = {}
            for ti in order:
                p, c = tile_pc(ti)
                ry = kpool.tile([P, FC], dt.float32, tag="ry", name="ry",
                                bufs=min(ntiles, 8))
                in0 = e1_t[p][:, c * NUC : (c + 1) * NUC, None].broadcast_to(
                    (P, NUC, VSUB)
                )
                in1 = e2_t[p][:, None, :].broadcast_to((P, NUC, VSUB))
                ryv = ry[:].rearrange("p (u v) -> p u v", v=VSUB)
                nc.vector._custom_dve(
                    op2, out=ryv, in0=in0, in1=in1,
                    s0=scal_t[p][:, 2:3], s1=MAGIC,
                )
                ry_store[ti] = ry

            # Phase S: sins + muls + out DMA, opc tiles first
            np1 = 0
            for ti in order:
                p, c = tile_pc(ti)
                st = scal_t[p]
                winfm_ap = (winfm_store[p][:] if c == 0
                            else winfm_store[p][:, ::-1])
                car = wpool.tile([P, FC], dt.float16, tag="car", name="car", bufs=4)
                ci = nc.scalar.activation(
                    car[:], ry_store[ti][:], AFT.Sin, scale=TWO_PI
                )
                add_dep_helper(ci.ins, exp_instrs[-1].ins, False, "act-table order")
                p1 = wpool.tile([P, FC], dt.float16, tag="p1", name="p1", bufs=4)
                if ti in opc_set:
                    nc.vector._custom_dve(
                        op5, out=p1[:], in0=car[:], in1=st[:, 26:27],
                        s0=st[:, 16 + c : 17 + c], s1=st[:, 20 + c : 21 + c],
                        imm2=A_COEF,
                    )
                else:
                    mod = wpool.tile([P, FC], dt.float16, tag="mod", name="mod",
                                     bufs=4)
                    mi = nc.scalar.activation(
                        mod[:], iota_t[:], AFT.Sin,
                        scale=st[:, 8 + c : 9 + c], bias=st[:, 12 + c : 13 + c],
                    )
                    add_dep_helper(mi.ins, exp_instrs[-1].ins, False,
                                   "act-table order")
                    eng = nc.vector if p1e_s[np1 % len(p1e_s)] == "v" else nc.gpsimd
                    np1 += 1
                    eng.tensor_mul(p1[:], car[:], mod[:])
                o = wpool.tile([P, FC], dt.float16, tag="o", name="o", bufs=4)
                eng = nc.vector if oe_s[ti % len(oe_s)] == "v" else nc.gpsimd
                eng.tensor_mul(o[:], p1[:], winfm_ap)
                nc.sync.dma_start(
                    out=out[p * P : (p + 1) * P, c * FC : (c + 1) * FC], in_=o[:]
                )
    nc.compile()
    return nc


def _host_params(theta_am, theta_fm):
    am_lo, am_hi = f32(math.log2(4.0)), f32(math.log2(16.0))
    fm_lo, fm_hi = f32(math.log2(0.5)), f32(math.log2(4.0))
    am = np.exp2(theta_am * (am_hi - am_lo) + am_lo).astype(f32)
    fm = np.exp2(theta_fm * (fm_hi - fm_lo) + fm_lo).astype(f32)

    fm_ln2 = (fm * f32(LN2)).astype(f32)
    c_phi = (f32(F0) / fm_ln2).astype(f32)
    c_hi = np.rint(c_phi.astype(np.float64)).astype(f32)
    c_lo = (c_phi - c_hi).astype(f32)
    am_half = (am * f32(0.5)).astype(f32)
    inv_s = (
        f32(1.0)
        / (np.abs(f32(SIGMA0 * BW_N) / fm).astype(f32) * f32(math.sqrt(2.0)))
    ).astype(f32)
    neg_inv2 = (-(inv_s * inv_s)).astype(f32)
    ln_fm = np.log(fm.astype(np.float64)).astype(f32)
    return fm, c_lo, am_half, neg_inv2, ln_fm, fm_ln2, c_phi


def plan_units(fm):
    """Sorted-batch grouping + needed (group, left-chunk) units.

    Returns (perm, units): perm sorts batches by descending fm;
    units = list of (group_index, left_chunk_j)."""
    perm = np.argsort(-fm, kind="stable")
    units = []
    for g in range(B // P):
        rows = perm[g * P : (g + 1) * P]
        fmin = float(fm[rows].min())
        radius = SUPPORT_T * math.sqrt(2.0) * (SIGMA0 * BW_N) / fmin
        for j in range(NLEFT):
            d_min = (NLEFT - 1 - j) * FC  # nearest |ws| of left chunk j
            if d_min < radius:
                units.append((g, j))
    return perm, units


def make_in_maps(theta_am, theta_fm):
    fm, c_lo, am_half, neg_inv2, ln_fm, fm_ln2, c_phi = _host_params(
        theta_am, theta_fm
    )
    perm, units = plan_units(fm)
    npair = (len(units) + NCORES - 1) // NCORES
    while len(units) < npair * NCORES:
        units.append(units[-1])  # padding; output ignored at assembly

    fm_ln2_64 = fm_ln2.astype(np.float64)
    c_phi_64 = c_phi.astype(np.float64)
    alpha_all = am_half.astype(np.float64) / SR

    ws_full = (np.arange(N, dtype=np.float64) - (N - 1) / 2.0)
    ws2_full = (ws_full ** 2).astype(f32)
    import ml_dtypes

    bf16 = ml_dtypes.bfloat16
    iota_row = np.arange(FC, dtype=np.float16)[None, :]
    v_idx = np.arange(VSUB, dtype=np.float64)

    core_units = [units[k::NCORES] for k in range(NCORES)]
    in_maps = []
    for k in range(NCORES):
        scal_k = np.zeros((npair * P, 32), dtype=f32)
        e1_k = np.zeros((npair * P, 2 * NUC), dtype=f32)
        e2_k = np.zeros((npair * P, VSUB), dtype=f32)
        ws2_k = np.zeros((npair, FC), dtype=bf16)
        for p, (g, j) in enumerate(core_units[k]):
            rows = perm[g * P : (g + 1) * P]
            sl = slice(p * P, (p + 1) * P)
            scal_k[sl, 2] = c_lo[rows]
            scal_k[sl, 4] = neg_inv2[rows]
            scal_k[sl, 5] = ln_fm[rows]
            scal_k[sl, 26] = B_COEF
            e2_k[sl] = np.exp(
                fm_ln2_64[rows, None] * v_idx[None, :] / SR
            ).astype(f32)
            s_left = j * FC
            ws2_k[p] = ws2_full[s_left : s_left + FC].astype(bf16)
            alpha = alpha_all[rows]
            for c, s_glob in ((0, s_left), (1, N - s_left - FC)):
                n0c = s_glob - N // 2
                u_idx = n0c + VSUB * np.arange(NUC, dtype=np.float64)
                e1_k[sl, c * NUC : (c + 1) * NUC] = (
                    c_phi_64[rows, None]
                    * np.exp(fm_ln2_64[rows, None] * u_idx[None, :] / SR)
                ).astype(f32)
                q0 = alpha * n0c
                qmid = q0 + alpha * (FC - 1) / 2.0
                r2 = np.round(2.0 * qmid)
                kp = r2 / 2.0
                sigma = 1.0 - 2.0 * (np.abs(r2).astype(np.int64) % 2)
                scal_k[sl, 8 + c] = (sigma * TWO_PI * alpha).astype(f32)
                scal_k[sl, 12 + c] = (sigma * TWO_PI * (q0 - kp)).astype(f32)
                scal_k[sl, 16 + c] = (S_NORM * sigma * alpha).astype(f32)
                scal_k[sl, 20 + c] = ((kp - q0) / alpha + 1.0).astype(f32)
        in_maps.append(
            {
                "scal": scal_k,
                "iota_row": iota_row,
                "e1": e1_k,
                "e2": e2_k,
                "ws2b": ws2_k,
            }
        )
    return in_maps, perm, core_units, npair


def build(npair):
    key = ("v4", npair, os.environ.get("CHIRP_OPC", ""),
           os.environ.get("CHIRP_P1E", ""), os.environ.get("CHIRP_OE", ""))
    if key not in _NC_CACHE:
        _NC_CACHE[key] = _build_nc_v4(npair)
    return _NC_CACHE[key]


def kernel(theta_am_hz_0to1, theta_fm_hz_0to1, seed=None, **_ignored):
    global LAST_RESULT
    from concourse.bass_utils import run_bass_kernel_spmd

    theta_am = np.asarray(theta_am_hz_0to1, dtype=f32)
    theta_fm = np.asarray(theta_fm_hz_0to1, dtype=f32)

    in_maps, perm, core_units, npair = make_in_maps(theta_am, theta_fm)
    nc = build(npair)
    global LAST_NC
    LAST_NC = nc

    trace = bool(int(os.environ.get("CHIRP_TRACE", "0")))
    res = run_bass_kernel_spmd(
        nc, in_maps, core_ids=list(range(NCORES)), trace=trace
    )
    LAST_RESULT = res

    full = np.zeros((B, N), dtype=f32)
    seen = set()
    for k in range(NCORES):
        o = res.results[k]["out"].astype(f32)
        for p, (g, j) in enumerate(core_units[k]):
            if (g, j) in seen:
                continue  # padding duplicate
            seen.add((g, j))
            rows = perm[g * P : (g + 1) * P]
            s = j * FC
            full[rows, s : s + FC] = o[p * P : (p + 1) * P, 0:FC]
            full[rows, N - s - FC : N - s] = o[p * P : (p + 1) * P, FC : 2 * FC]
    return np.ascontiguousarray(full.reshape(B, 1, N))


# revision 15
# speedup vs baseline: 1.3104x; 1.1894x over previous
"""ChirpletSynth Trainium2 kernel (v4: sorted-batch zero-window skipping).

out[b, n] = sin(2*pi*phi) * fm * exp(-(ws*inv)^2) * sin(2*pi*am*0.5*t)
  phi = (F0/(fm*ln2)) * (2^(fm*t) - 1)

The gaussian window exp(-(ws/(std*sqrt2))^2), std = 4410/fm samples, is
negligible beyond |ws| > 4.67*std, so most (batch, chunk) tiles far from
the center are exact zeros. Batches are sorted by fm into 2 groups of
128; for each group only the mirror-pairs of 2048-sample chunks that
intersect the group's support are computed. The needed (pair, group)
units are distributed round-robin over 8 cores; every core runs the
same program over NPAIR slot-pairs, with per-slot constants (scal, e1,
e2, ws2, modulator phases) supplied in the per-core input maps. Host
assembles slots back into the full [B, N] output (and zeros the rest).

Per slot-pair p (tiles L and R, R mirrors L so winfm is reused reversed):
  ACT : winfm = Exp(neg_inv2*ws2 + ln_fm)       (ws2 broadcast, bf16)
  DVE : ry    = red(E1*E2 - c_lo)               (fp32 custom, separable exp)
  ACT : car   = Sin(2pi*ry) -> fp16
  modulator: ACT Sin(sc*iota+bi) + fp16 mul, or fused DVE custom
             car*sinpoly5(s0*(J-s1)) (scan-index J, C3-spilled coeff)
  DVE/Pool: o = p1 * winfm
All Exp activations precede all Sin ones (one act-table switch).
"""

import math
import os

import numpy as np

P = 128
B = 256
N = 65536
NCORES = 8

SR = 44100.0
F0 = 440.0
SIGMA0 = 0.1
BW_N = 44100
LN2 = math.log(2.0)
TWO_PI = 2.0 * math.pi
MAGIC = 12582912.0  # 1.5 * 2**23

VSUB = 128
FC = 2048
NUC = FC // VSUB  # 16
NLEFT = (N // 2) // FC  # 16 left-half chunks; right half mirrors them
SUPPORT_T = float(os.environ.get("CHIRP_T", "3.3"))  # window cutoff, in stds/sqrt2

# deg-5 minimax of sin(2*pi*y) on |y| <= 0.4365, factored with s^5 = c5:
#   sin(2*pi*y) ~= y'*((y'^2 + B)*y'^2 + A),  y' = s*y
_C1, _C3, _C5 = 6.236727, -39.32464819, 59.29172001
S_NORM = _C5 ** 0.2
A_COEF = _C1 / S_NORM
B_COEF = _C3 / S_NORM ** 3

f32 = np.float32

_OP2 = None
_OP5 = None
_NC_CACHE = {}
LAST_RESULT = None
LAST_NC = None


def _register_chirp_exp_op():
    """w = in0*in1 - s0 ; out = w - round(w) (round via magic constant s1)."""
    global _OP2
    if _OP2 is not None:
        return _OP2
    import concourse.dve_ops as D
    from concourse.dve_spec import Spec, Src0, Src1, C0, C1, lower, _has_src1
    from concourse.dve_uop import DveOpSpec

    name = "CHIRP_EXP_RED"
    for op in D.OPS:
        if op.name == name:
            _OP2 = op
            return op

    w = Src0 * Src1 - C0
    body = w - ((w + C1) - C1)

    def _ref(in0, in1, s0, s1, imm2):
        ww = (in0.astype(np.float32) * in1.astype(np.float32)).astype(np.float32)
        ww = (ww - s0).astype(np.float32)
        u = (ww + np.float32(s1)).astype(np.float32)
        r = (u - np.float32(s1)).astype(np.float32)
        return (ww - r).astype(np.float32)

    spec = Spec(body=body, reference=_ref)
    row = D._CUSTOM_DVE_ROW_BASE + len(D.OPS)
    assert row < 0x20, "custom-DVE opcode rows exhausted"
    D._SUB_OPCODE_FOR_NAME[name] = row
    shas = {}
    for ver in ("v3", "v4"):
        tmp = DveOpSpec(
            name=name, opcode=row, uops=lower(spec, ver=ver), rd1_en=_has_src1(spec)
        )
        shas[ver] = tmp.sha(ver)
    op = D.DveOp(name, spec, subdim=False, uops_sha=shas)
    D.OPS.append(op)
    D.CUSTOM_DVE_SPECS[name] = spec
    _OP2 = op
    return op


def _register_chirp_mod5_op():
    """p1 = in0 * sinpoly5(s0*(J - s1)), J(k) = k+1 via an ADD-scan with the
    offset folded into the scan init. Deg-5 coefficient b rides the C3 spill
    (in1, read once at element 0); a is the imm2 literal."""
    global _OP5
    if _OP5 is not None:
        return _OP5
    import concourse.dve_ops as D
    from concourse.dve_spec import (
        Spec, Src0, C0, C1, C2, C3, Zero, One, scan, lower,
        _has_src1, _spill_c3_to_src1, AluOp,
    )
    from concourse.dve_uop import DveOpSpec

    name = "CHIRP_MOD5"
    for op in D.OPS:
        if op.name == name:
            _OP5 = op
            return op

    J = scan(AluOp.ADD, One, init=Zero - C1)  # J(k) = (k+1) - s1
    y = J * C0
    t = y * y
    m = t + C3
    n = m * t
    n2 = n + C2
    r = n2 * y
    body = _spill_c3_to_src1(r * Src0)

    def _ref(in0, in1, s0, s1, imm2):
        k = np.arange(in0.shape[-1], dtype=np.float32)
        J_ = (k + np.float32(1.0)) - np.float32(s1)
        y_ = (J_ * np.float32(s0)).astype(np.float32)
        t_ = y_ * y_
        b = np.float32(in1.reshape(in1.shape[0], -1)[:, 0:1])
        return (((t_ + b) * t_ + np.float32(imm2)) * y_ * in0.astype(np.float32)
                ).astype(np.float32)

    spec = Spec(body=body, reference=_ref)
    row = D._CUSTOM_DVE_ROW_BASE + len(D.OPS)
    assert row < 0x20, "custom-DVE opcode rows exhausted"
    D._SUB_OPCODE_FOR_NAME[name] = row
    shas = {}
    for ver in ("v3", "v4"):
        tmp = DveOpSpec(
            name=name, opcode=row, uops=lower(spec, ver=ver), rd1_en=_has_src1(spec)
        )
        shas[ver] = tmp.sha(ver)
    op = D.DveOp(name, spec, subdim=False, uops_sha=shas)
    D.OPS.append(op)
    D.CUSTOM_DVE_SPECS[name] = spec
    _OP5 = op
    return op


def _build_nc_v4(npair):
    """One program with `npair` slot-pairs (2 tiles each). Per-slot data
    (scal/e1/e2/ws2) comes from DRAM arrays indexed by slot."""
    import concourse.bass as bass  # noqa: F401
    import concourse.mybir as mybir
    from concourse import bacc
    from concourse.tile import TileContext, add_dep_helper

    AFT = mybir.ActivationFunctionType
    dt = mybir.dt
    op2 = _register_chirp_exp_op()
    op5 = _register_chirp_mod5_op()

    ntiles = 2 * npair
    opc = os.environ.get("CHIRP_OPC", "0,1,2")
    opc_set = {int(x) for x in opc.split(",") if x != ""} & set(range(ntiles))
    oe_s = os.environ.get("CHIRP_OE", "gggvvvvv")
    p1e_s = os.environ.get("CHIRP_P1E", "gvvvv")

    nc = bacc.Bacc(None, target_bir_lowering=False, debug=False)
    scal = nc.declare_dram_parameter("scal", [npair * P, 32], dt.float32,
                                     isOutput=False)
    iota_row = nc.declare_dram_parameter("iota_row", [1, FC], dt.float16,
                                         isOutput=False)
    e1 = nc.declare_dram_parameter("e1", [npair * P, 2 * NUC], dt.float32,
                                   isOutput=False)
    e2 = nc.declare_dram_parameter("e2", [npair * P, VSUB], dt.float32,
                                   isOutput=False)
    ws2b = nc.declare_dram_parameter("ws2b", [npair, FC], dt.bfloat16,
                                     isOutput=False)
    out = nc.declare_dram_parameter("out", [npair * P, 2 * FC], dt.float16,
                                    isOutput=True)

    with TileContext(nc) as tc:
        with (
            tc.tile_pool(name="consts", bufs=1) as cpool,
            tc.tile_pool(name="keep", bufs=8) as kpool,
            tc.tile_pool(name="work", bufs=4) as wpool,
        ):
            # const DMAs on SP queue; op2 + exp inputs first, iota last
            scal_t, e1_t, e2_t, ws2_t = [], [], [], []
            for p in range(npair):
                st = cpool.tile([P, 32], dt.float32, tag=f"scal{p}", name=f"scal{p}")
                nc.sync.dma_start(out=st[:], in_=scal[p * P : (p + 1) * P, :])
                scal_t.append(st)
            for p in range(npair):
                wb = cpool.tile([P, FC], dt.bfloat16, tag=f"ws2b{p}",
                                name=f"ws2b{p}")
                nc.sync.dma_start(
                    out=wb[:], in_=ws2b[p : p + 1, :].to_broadcast((P, FC))
                )
                ws2_t.append(wb)
            for p in range(npair):
                e1g = cpool.tile([P, 2 * NUC], dt.float32, tag=f"e1{p}",
                                 name=f"e1{p}")
                nc.sync.dma_start(out=e1g[:], in_=e1[p * P : (p + 1) * P, :])
                e1_t.append(e1g)
                e2g = cpool.tile([P, VSUB], dt.float32, tag=f"e2{p}", name=f"e2{p}")
                nc.sync.dma_start(out=e2g[:], in_=e2[p * P : (p + 1) * P, :])
                e2_t.append(e2g)
            iota_t = cpool.tile([P, FC], dt.float16, tag="iota", name="iota")
            nc.sync.dma_start(
                out=iota_t[:], in_=iota_row[0:1, :].to_broadcast((P, FC))
            )

            # Phase W: all Exps first (one act-table phase)
            winfm_store = []
            exp_instrs = []
            for p in range(npair):
                winfm = kpool.tile([P, FC], dt.float16, tag="winfm", name="winfm",
                                   bufs=npair)
                wi = nc.scalar.activation(
                    winfm[:], ws2_t[p][:], AFT.Exp,
                    scale=scal_t[p][:, 4:5], bias=scal_t[p][:, 5:6],
                )
                exp_instrs.append(wi)
                winfm_store.append(winfm)

            # tiles: ti = 2*p + side (side 0 = left chunk, 1 = mirrored right)
            def tile_pc(ti):
                return ti // 2, ti % 2

            # Phase D: carrier range reductions, opc tiles' ry first
            order = sorted(range(ntiles), key=lambda t: (t not in opc_set, t))
            ry_store = {}
            for ti in order:
                p, c = tile_pc(ti)
                ry = kpool.tile([P, FC], dt.float32, tag="ry", name="ry",
                                bufs=min(ntiles, 8))
                in0 = e1_t[p][:, c * NUC : (c + 1) * NUC, None].broadcast_to(
                    (P, NUC, VSUB)
                )
                in1 = e2_t[p][:, None, :].broadcast_to((P, NUC, VSUB))
                ryv = ry[:].rearrange("p (u v) -> p u v", v=VSUB)
                nc.vector._custom_dve(
                    op2, out=ryv, in0=in0, in1=in1,
                    s0=scal_t[p][:, 2:3], s1=MAGIC,
                )
                ry_store[ti] = ry

            # Phase S: sins + muls + out DMA, opc tiles first
            np1 = 0
            for ti in order:
                p, c = tile_pc(ti)
                st = scal_t[p]
                winfm_ap = (winfm_store[p][:] if c == 0
                            else winfm_store[p][:, ::-1])
                car = wpool.tile([P, FC], dt.float16, tag="car", name="car", bufs=4)
                ci = nc.scalar.activation(
                    car[:], ry_store[ti][:], AFT.Sin, scale=TWO_PI
                )
                add_dep_helper(ci.ins, exp_instrs[-1].ins, False, "act-table order")
                p1 = wpool.tile([P, FC], dt.float16, tag="p1", name="p1", bufs=4)
                if ti in opc_set:
                    nc.vector._custom_dve(
                        op5, out=p1[:], in0=car[:], in1=st[:, 26:27],
                        s0=st[:, 16 + c : 17 + c], s1=st[:, 20 + c : 21 + c],
                        imm2=A_COEF,
                    )
                else:
                    mod = wpool.tile([P, FC], dt.float16, tag="mod", name="mod",
                                     bufs=4)
                    mi = nc.scalar.activation(
                        mod[:], iota_t[:], AFT.Sin,
                        scale=st[:, 8 + c : 9 + c], bias=st[:, 12 + c : 13 + c],
                    )
                    add_dep_helper(mi.ins, exp_instrs[-1].ins, False,
                                   "act-table order")
                    eng = nc.vector if p1e_s[np1 % len(p1e_s)] == "v" else nc.gpsimd
                    np1 += 1
                    eng.tensor_mul(p1[:], car[:], mod[:])
                o = wpool.tile([P, FC], dt.float16, tag="o", name="o", bufs=4)
                eng = nc.vector if oe_s[ti % len(oe_s)] == "v" else nc.gpsimd
                eng.tensor_mul(o[:], p1[:], winfm_ap)
                nc.sync.dma_start(
                    out=out[p * P : (p + 1) * P, c * FC : (c + 1) * FC], in_=o[:]
                )
    nc.compile()
    return nc


def _host_params(theta_am, theta_fm):
    am_lo, am_hi = f32(math.log2(4.0)), f32(math.log2(16.0))
    fm_lo, fm_hi = f32(math.log2(0.5)), f32(math.log2(4.0))
    am = np.exp2(theta_am * (am_hi - am_lo) + am_lo).astype(f32)
    fm = np.exp2(theta_fm * (fm_hi - fm_lo) + fm_lo).astype(f32)

    fm_ln2 = (fm * f32(LN2)).astype(f32)
    c_phi = (f32(F0) / fm_ln2).astype(f32)
    c_hi = np.rint(c_phi.astype(np.float64)).astype(f32)
    c_lo = (c_phi - c_hi).astype(f32)
    am_half = (am * f32(0.5)).astype(f32)
    inv_s = (
        f32(1.0)
        / (np.abs(f32(SIGMA0 * BW_N) / fm).astype(f32) * f32(math.sqrt(2.0)))
    ).astype(f32)
    neg_inv2 = (-(inv_s * inv_s)).astype(f32)
    ln_fm = np.log(fm.astype(np.float64)).astype(f32)
    return fm, c_lo, am_half, neg_inv2, ln_fm, fm_ln2, c_phi


def plan_units(fm):
    """Sorted-batch grouping + needed (group, left-chunk) units.

    Returns (perm, units): perm sorts batches by descending fm;
    units = list of (group_index, left_chunk_j)."""
    perm = np.argsort(-fm, kind="stable")
    units = []
    for g in range(B // P):
        rows = perm[g * P : (g + 1) * P]
        fmin = float(fm[rows].min())
        radius = SUPPORT_T * math.sqrt(2.0) * (SIGMA0 * BW_N) / fmin
        for j in range(NLEFT):
            d_min = (NLEFT - 1 - j) * FC  # nearest |ws| of left chunk j
            if d_min < radius:
                units.append((g, j))
    return perm, units


def make_in_maps(theta_am, theta_fm):
    fm, c_lo, am_half, neg_inv2, ln_fm, fm_ln2, c_phi = _host_params(
        theta_am, theta_fm
    )
    perm, units = plan_units(fm)
    npair = (len(units) + NCORES - 1) // NCORES
    while len(units) < npair * NCORES:
        units.append(units[-1])  # padding; output ignored at assembly

    fm_ln2_64 = fm_ln2.astype(np.float64)
    c_phi_64 = c_phi.astype(np.float64)
    alpha_all = am_half.astype(np.float64) / SR

    ws_full = (np.arange(N, dtype=np.float64) - (N - 1) / 2.0)
    ws2_full = (ws_full ** 2).astype(f32)
    import ml_dtypes

    bf16 = ml_dtypes.bfloat16
    iota_row = np.arange(FC, dtype=np.float16)[None, :]
    v_idx = np.arange(VSUB, dtype=np.float64)

    core_units = [units[k::NCORES] for k in range(NCORES)]
    in_maps = []
    for k in range(NCORES):
        scal_k = np.zeros((npair * P, 32), dtype=f32)
        e1_k = np.zeros((npair * P, 2 * NUC), dtype=f32)
        e2_k = np.zeros((npair * P, VSUB), dtype=f32)
        ws2_k = np.zeros((npair, FC), dtype=bf16)
        for p, (g, j) in enumerate(core_units[k]):
            rows = perm[g * P : (g + 1) * P]
            sl = slice(p * P, (p + 1) * P)
            scal_k[sl, 2] = c_lo[rows]
            scal_k[sl, 4] = neg_inv2[rows]
            scal_k[sl, 5] = ln_fm[rows]
            scal_k[sl, 26] = B_COEF
            e2_k[sl] = np.exp(
                fm_ln2_64[rows, None] * v_idx[None, :] / SR
            ).astype(f32)
            s_left = j * FC
            ws2_k[p] = ws2_full[s_left : s_left + FC].astype(bf16)
            alpha = alpha_all[rows]
            for c, s_glob in ((0, s_left), (1, N - s_left - FC)):
                n0c = s_glob - N // 2
                u_idx = n0c + VSUB * np.arange(NUC, dtype=np.float64)
                e1_k[sl, c * NUC : (c + 1) * NUC] = (
                    c_phi_64[rows, None]
                    * np.exp(fm_ln2_64[rows, None] * u_idx[None, :] / SR)
                ).astype(f32)
                q0 = alpha * n0c
                qmid = q0 + alpha * (FC - 1) / 2.0
                r2 = np.round(2.0 * qmid)
                kp = r2 / 2.0
                sigma = 1.0 - 2.0 * (np.abs(r2).astype(np.int64) % 2)
                scal_k[sl, 8 + c] = (sigma * TWO_PI * alpha).astype(f32)
                scal_k[sl, 12 + c] = (sigma * TWO_PI * (q0 - kp)).astype(f32)
                scal_k[sl, 16 + c] = (S_NORM * sigma * alpha).astype(f32)
                scal_k[sl, 20 + c] = ((kp - q0) / alpha + 1.0).astype(f32)
        in_maps.append(
            {
                "scal": scal_k,
                "iota_row": iota_row,
                "e1": e1_k,
                "e2": e2_k,
                "ws2b": ws2_k,
            }
        )
    return in_maps, perm, core_units, npair


def build(npair):
    key = ("v4", npair, os.environ.get("CHIRP_OPC", ""),
           os.environ.get("CHIRP_P1E", ""), os.environ.get("CHIRP_OE", ""))
    if key not in _NC_CACHE:
        _NC_CACHE[key] = _build_nc_v4(npair)
    return _NC_CACHE[key]


def kernel(theta_am_hz_0to1, theta_fm_hz_0to1, seed=None, **_ignored):
    global LAST_RESULT
    from concourse.bass_utils import run_bass_kernel_spmd

    theta_am = np.asarray(theta_am_hz_0to1, dtype=f32)
    theta_fm = np.asarray(theta_fm_hz_0to1, dtype=f32)

    in_maps, perm, core_units, npair = make_in_maps(theta_am, theta_fm)
    nc = build(npair)
    global LAST_NC
    LAST_NC = nc

    trace = bool(int(os.environ.get("CHIRP_TRACE", "0")))
    res = run_bass_kernel_spmd(
        nc, in_maps, core_ids=list(range(NCORES)), trace=trace
    )
    LAST_RESULT = res

    full = np.zeros((B, N), dtype=f32)
    seen = set()
    for k in range(NCORES):
        o = res.results[k]["out"].astype(f32)
        for p, (g, j) in enumerate(core_units[k]):
            if (g, j) in seen:
                continue  # padding duplicate
            seen.add((g, j))
            rows = perm[g * P : (g + 1) * P]
            s = j * FC
            full[rows, s : s + FC] = o[p * P : (p + 1) * P, 0:FC]
            full[rows, N - s - FC : N - s] = o[p * P : (p + 1) * P, FC : 2 * FC]
    return np.ascontiguousarray(full.reshape(B, 1, N))
